# revision 30
# baseline (speedup 1.0000x reference)
"""Trainium2 Bass kernel for LGAttention (global MHA + windowed local MHA).

Sharding: one attention head per NeuronCore (8 heads, 8 cores), SPMD.

v2 design (vs baseline): the kernel is paced by the ScalarE exp of the full
attention matrix; everything else hides under it.
  - Global S matmuls (contraction hd=48) are row-packed: qT/kT live at
    partitions 0-47 AND (duplicated via SBUF->SBUF DMA) 64-111; k-blocks of
    alternating parity use alternating row groups so consecutive S matmuls run
    concurrently in the PE array.
  - PV matmuls are col-packed: v_aug is 64 wide (48 v + pad + ones col at 63,
    so the softmax denominator lands on out-partition 63/127); two q-ranges
    are processed per pair with accumulators at out partitions 0-63 / 64-127
    of one PSUM bank (memset + start=False accumulation).
  - exp is one ACT instruction per k-block covering both q-ranges (896 wide).
  - Local branch: window pairs stacked at partitions 0-48 / 64-112
    (diagonal-tiled S and PV), exp batched 16 windows per instruction.
  - Projections row-packed (outT for even/odd ranges at partitions 0-47 /
    64-111, proj weights duplicated) and interleaved as PE filler.
Host: divides by the denominators, un-permutes the local branch, sums the
8 per-head partials, adds biases (same contract as baseline).
"""

import sys

sys.path.insert(0, "/opt/trn_rl_repo")

from collections import deque

import numpy as np
import ml_dtypes

import concourse.bass as bass
import concourse.mybir as mybir
import concourse.tile as tile
from concourse import bacc, bass_utils

BF16 = mybir.dt.bfloat16
F32 = mybir.dt.float32

B, N, C = 2, 3136, 384
H, HD, WS = 8, 48, 7
NT = B * N            # 6272 tokens total
WT = WS * WS          # 49 tokens per window
Q = 448               # q-range width; 7 ranges per batch
VS = 64               # v_aug column stride: 48 v + 15 pad + 1 ones (col 63)
SCALE = float(HD) ** -0.5
NKB = 25              # k-blocks per batch (24x128 + 64)
EXP = mybir.ActivationFunctionType.Exp

# global q-range pairs: (batch_A, range_A, batch_B, range_B)
PAIRS = [(0, 0, 0, 1), (0, 2, 0, 3), (0, 4, 0, 5),
         (1, 0, 1, 1), (1, 2, 1, 3), (1, 4, 1, 5),
         (0, 6, 1, 6)]


def build_program():
    nc = bacc.Bacc(
        "TRN2",
        target_bir_lowering=False,
        debug=False,
        enable_asserts=False,
        num_devices=8,
    )

    din = {}
    for name, shape in [
        ("xT", (C, NT)), ("winT", (C, NT)),
        ("gwqk", (C, 112)), ("gwv", (C, HD)), ("gwp", (HD, C)),
        ("lwqk", (C, 112)), ("lwv", (C, HD)), ("lwp", (HD, C)),
    ]:
        din[name] = nc.dram_tensor(name, list(shape), BF16, kind="ExternalInput").ap()

    dout = {}
    for name, shape in [
        ("g_out", (NT, C)), ("l_out", (NT, C)),
        ("g_den", (1, NT)), ("l_den", (1, NT)),
    ]:
        dout[name] = nc.dram_tensor(name, list(shape), F32, kind="ExternalOutput").ap()

    with tile.TileContext(nc) as tc:
        _emit(tc, nc, din, dout)

    nc.compile()
    return nc


def _emit(tc, nc, din, dout):
    from contextlib import ExitStack

    ctx = ExitStack()
    with ctx:
        persist = ctx.enter_context(tc.tile_pool(name="persist", bufs=1))
        psum = ctx.enter_context(tc.tile_pool(name="psum", bufs=1, space="PSUM"))
        work = ctx.enter_context(tc.tile_pool(name="work", bufs=1))

        # ---- persistent tiles ----
        xt = [persist.tile([128, NT], BF16, name=f"xt{c}") for c in range(3)]
        wt = [persist.tile([128, NT], BF16, name=f"wt{c}") for c in range(3)]
        gwqk = persist.tile([128, 3 * 112], BF16, name="gwqk")
        lwqk = persist.tile([128, 3 * 112], BF16, name="lwqk")
        gwv = persist.tile([128, 3 * 48], BF16, name="gwv")
        lwv = persist.tile([128, 3 * 48], BF16, name="lwv")
        gwp_d = persist.tile([128, C], BF16, name="gwp_d")
        lwp_d = persist.tile([128, C], BF16, name="lwp_d")
        # q/k in transposed layout, duplicated on partitions 0-47 and 64-111:
        # cols [0,NT) = qT (token t at col t), cols [NT,2NT) = kT, + 64 pad
        # cols (local S reads 64-wide lhsT slices that overrun the last window)
        QKg = persist.tile([128, 2 * NT + 64], BF16, name="QKg")
        QKl = persist.tile([128, 2 * NT + 64], BF16, name="QKl")
        g_vaug = persist.tile([128, 2 * NKB * VS], BF16, name="g_vaug")
        l_vaug = persist.tile([128, 64 * VS], BF16, name="l_vaug")
        # outT: rows 0-47 = even q-range (A), rows 64-111 = odd q-range (B);
        # pair p occupies cols [448p, 448p+448)
        g_outT = persist.tile([128, 7 * Q], BF16, name="g_outT")
        # local: rows 0-47 = even window, 64-111 = odd window; win pair s at
        # cols [49s, 49s+49)
        l_outT = persist.tile([128, 64 * WT], BF16, name="l_outT")

        # ---- weight + input DMAs ----
        for c in range(3):
            nc.scalar.dma_start(gwqk[:, c * 112:(c + 1) * 112], din["gwqk"][c * 128:(c + 1) * 128, :])
            nc.scalar.dma_start(lwqk[:, c * 112:(c + 1) * 112], din["lwqk"][c * 128:(c + 1) * 128, :])
            nc.scalar.dma_start(gwv[:, c * 48:(c + 1) * 48], din["gwv"][c * 128:(c + 1) * 128, :])
            nc.scalar.dma_start(lwv[:, c * 48:(c + 1) * 48], din["lwv"][c * 128:(c + 1) * 128, :])
        nc.scalar.dma_start(gwp_d[0:48, :], din["gwp"][:, :])
        nc.scalar.dma_start(gwp_d[64:112, :], din["gwp"][:, :])
        nc.scalar.dma_start(lwp_d[0:48, :], din["lwp"][:, :])
        nc.scalar.dma_start(lwp_d[64:112, :], din["lwp"][:, :])

        # dummy exp to pull the ACT table load into the DMA phase
        dmy = work.tile([1, 16], F32, name="dmy", tag="dmy", bufs=1)
        dmyo = work.tile([1, 16], BF16, name="dmyo", tag="dmy2", bufs=1)
        nc.vector.memset(dmy[:, :], 0.0)
        nc.scalar.activation(dmyo[:, :], dmy[:, :], EXP, scale=SCALE)

        # input DMAs: one half-tensor (contiguous DRAM rows) per c-chunk,
        # spread across three DMA queues (sync / gpsimd / vector) so the
        # first qk-proj can start after ~one transfer
        qengs = [nc.sync, nc.scalar, nc.sync]
        for lo in range(0, N, 896):      # fine-grained chunks for fast ramp
            for c in range(3):
                hi = min(lo + 896, N)
                qengs[c].dma_start(xt[c][:, lo:hi], din["xT"][c * 128:(c + 1) * 128, lo:hi])
        for c in range(3):
            qengs[c].dma_start(xt[c][:, N:NT], din["xT"][c * 128:(c + 1) * 128, N:NT])
        for lo in (0, N):
            for c in range(3):
                qengs[c].dma_start(wt[c][:, lo:lo + N], din["winT"][c * 128:(c + 1) * 128, lo:lo + N])

        # ---- pad/ones init ----
        for vaug, nsl in ((g_vaug, 2 * NKB), (l_vaug, 64)):
            v3 = vaug[:, :].rearrange("p (s k) -> p s k", k=VS)
            nc.vector.memset(v3[:, :, 48:VS], 0.0)
            nc.vector.memset(v3[:, :, 63:VS], 1.0)
        nc.vector.memset(QKg[:, 2 * NT:], 0.0)
        nc.vector.memset(QKl[:, 2 * NT:], 0.0)

        # ---- qk projection emitter (writes qT to rows 0-47, kT to 64-111) ----
        def qk_proj(qb, wqk, QK):
            t0 = qb * Q
            ps = psum.tile([112, Q], F32, name="pqk", tag="pmix", bufs=2)
            src = xt if QK is QKg else wt
            for c in range(3):
                nc.tensor.matmul(ps[:, :], wqk[:, c * 112:(c + 1) * 112],
                                 src[c][:, t0:t0 + Q], start=(c == 0), stop=(c == 2))
            nc.vector.tensor_copy(QK[0:48, t0:t0 + Q], ps[0:48, :])
            nc.vector.tensor_copy(QK[64:112, NT + t0:NT + t0 + Q], ps[64:112, :])

        # duplicate q (lo->hi) and k (hi->lo) for one batch via SBUF->SBUF DMA
        def qk_dup(QK, b):
            t0 = b * N
            nc.sync.dma_start(QK[64:112, t0:t0 + N], QK[0:48, t0:t0 + N])
            nc.sync.dma_start(QK[0:48, NT + t0:NT + t0 + N], QK[64:112, NT + t0:NT + t0 + N])

        # ---- global v projection: one 128-token block, token-major ----
        def gv_block(bl):
            t0 = (bl // NKB) * N + (bl % NKB) * 128
            sz = 64 if bl % NKB == NKB - 1 else 128
            pv = psum.tile([128, 48], F32, name="pv", tag="pmix", bufs=2)
            for c in range(3):
                nc.tensor.matmul(pv[0:sz, :], xt[c][:, t0:t0 + sz],
                                 gwv[:, c * 48:(c + 1) * 48], start=(c == 0), stop=(c == 2))
            nc.vector.tensor_copy(g_vaug[0:sz, bl * VS:bl * VS + 48], pv[0:sz, :])

        # ---- local v projection: one window pair (A rows 0-48, B rows 64-112) ----
        def lv_pair(s):
            tA = (2 * s) * WT
            tB = (2 * s + 1) * WT
            pvl = psum.tile([128, 48], F32, name="pvl", tag="pmix", bufs=2)
            nc.vector.memset(pvl[:, :], 0.0)
            for c in range(3):
                nc.tensor.matmul(pvl[0:WT, :], wt[c][:, tA:tA + WT],
                                 lwv[:, c * 48:(c + 1) * 48], start=False, stop=(c == 2),
                                 skip_group_check=True)
            for c in range(3):
                nc.tensor.matmul(pvl[64:64 + WT, :], wt[c][:, tB:tB + WT],
                                 lwv[:, c * 48:(c + 1) * 48], start=False, stop=(c == 2),
                                 skip_group_check=True)
            nc.vector.tensor_copy(l_vaug[:, s * VS:s * VS + 48], pvl[:, :])

        # ---- one local region: 8 window pairs (16 windows) ----
        def local_region(r):
            psl = psum.tile([128, 8 * WT], F32, name="psl", tag="pmix", bufs=2)
            for u in range(8):
                s = 8 * r + u
                tA, tB = (2 * s) * WT, (2 * s + 1) * WT
                # lhsT free padded to 64 (overruns into next window / pad cols)
                nc.tensor.matmul(psl[0:64, u * WT:(u + 1) * WT],
                                 QKl[0:48, NT + tA:NT + tA + 64],
                                 QKl[0:48, tA:tA + WT], start=True, stop=True)
                nc.tensor.matmul(psl[64:128, u * WT:(u + 1) * WT],
                                 QKl[64:112, NT + tB:NT + tB + 64],
                                 QKl[64:112, tB:tB + WT], start=True, stop=True)
            exl = work.tile([128, 8 * WT], BF16, name="exl", tag="exl", bufs=3)
            nc.scalar.activation(exl[:, :], psl[:, :], EXP, scale=SCALE)
            pol = psum.tile([128, 8 * WT], F32, name="pol", tag="pmix", bufs=2)
            for u in range(8):
                s = 8 * r + u
                nc.tensor.matmul(pol[0:64, u * WT:(u + 1) * WT],
                                 l_vaug[0:WT, s * VS:(s + 1) * VS],
                                 exl[0:WT, u * WT:(u + 1) * WT], start=True, stop=True)
                nc.tensor.matmul(pol[64:128, u * WT:(u + 1) * WT],
                                 l_vaug[64:64 + WT, s * VS:(s + 1) * VS],
                                 exl[64:64 + WT, u * WT:(u + 1) * WT], start=True, stop=True)
            nc.vector.tensor_copy(l_outT[0:48, r * 392:(r + 1) * 392], pol[0:48, :])
            nc.vector.tensor_copy(l_outT[64:112, r * 392:(r + 1) * 392], pol[64:112, :])
            dnA = work.tile([32, 392], F32, name="dnA", tag="dn", bufs=4)
            dnB = work.tile([32, 392], F32, name="dnB", tag="dn", bufs=4)
            nc.vector.tensor_copy(dnA[:, :], pol[32:64, :])
            nc.vector.tensor_copy(dnB[:, :], pol[96:128, :])
            ld = dout["l_den"].rearrange("o (a b t) -> o a b t", b=2, t=WT)
            nc.gpsimd.dma_start(ld[0:1, 8 * r:8 * r + 8, 0, :],
                              dnA[31:32, :].rearrange("o (a t) -> o a t", t=WT))
            nc.gpsimd.dma_start(ld[0:1, 8 * r:8 * r + 8, 1, :],
                              dnB[31:32, :].rearrange("o (a t) -> o a t", t=WT))

        # ---- projection block emitters ----
        def gproj_block(p, t):
            # one 112-token block of pair p, both ranges (adjacent row-tiled
            # matmuls on row groups 0 / 64 -> concurrent)
            bA, rA, bB, rB = PAIRS[p]
            col = p * Q + t * 112
            pps = []
            for hi, q0 in ((0, bA * N + rA * Q), (1, bB * N + rB * Q)):
                rg = 64 * hi
                pp = psum.tile([112, C], F32, name="pp", tag="pmix", bufs=2)
                nc.tensor.matmul(pp[:, :], g_outT[rg:rg + 48, col:col + 112],
                                 gwp_d[rg:rg + 48, :], start=True, stop=True)
                pps.append((pp, q0))
            for pp, q0 in pps:
                sp = work.tile([112, C], F32, name="sp", tag="sp", bufs=5)
                nc.vector.tensor_copy(sp[:, :], pp[:, :])
                nc.sync.dma_start(dout["g_out"][q0 + t * 112:q0 + t * 112 + 112, :], sp[:, :])

        def lproj_block(s0):
            # windows 2*s0 .. 2*s0+3 (both parities, row-tiled concurrent)
            col = s0 * WT
            pps = []
            for hi in (0, 1):
                rg = 64 * hi
                pp = psum.tile([98, C], F32, name="ppl", tag="pmix", bufs=2)
                nc.tensor.matmul(pp[:, :], l_outT[rg:rg + 48, col:col + 98],
                                 lwp_d[rg:rg + 48, :], start=True, stop=True)
                pps.append((pp, hi))
            for pp, hi in pps:
                sp = work.tile([98, C], F32, name="spl", tag="sp", bufs=5)
                nc.vector.tensor_copy(sp[:, :], pp[:, :])
                r0 = (2 * s0 + hi) * WT
                r1 = (2 * (s0 + 1) + hi) * WT
                nc.sync.dma_start(dout["l_out"][r0:r0 + WT, :], sp[0:WT, :])
                nc.gpsimd.dma_start(dout["l_out"][r1:r1 + WT, :], sp[WT:2 * WT, :])

        # ---- HAM pre-warm: dense dummy matmuls while input DMAs stream ----
        # (the PE is otherwise idle here; ~6us of back-to-back matmuls pushes
        # the activity monitor to K=8/8 before the real work begins)
        pwm = psum.tile([112, 448], F32, name="pwm", tag="pmix", bufs=2)
        for _ in range(16):
            nc.tensor.matmul(pwm[:, :336], gwqk[:, 0:112], gwqk[:, 0:336],
                             start=True, stop=True)

        # ---- b0 qk projection + per-chunk dup ----
        def gqk_item(qb):
            qk_proj(qb, gwqk, QKg)
            t0 = qb * Q
            nc.sync.dma_start(QKg[64:112, t0:t0 + Q], QKg[0:48, t0:t0 + Q])
            nc.sync.dma_start(QKg[0:48, NT + t0:NT + t0 + Q], QKg[64:112, NT + t0:NT + t0 + Q])

        # only qb0-1 gate pair 0's first k-blocks; qb2-6 stream in as early
        # fillers so the exp pipeline starts ~15us sooner
        for qb in range(2):
            gqk_item(qb)

        # ---- filler queue for the global loop ----
        fillers = deque()
        # interleave the deferred b0 qk blocks with the first v blocks so
        # PV(j)'s v-block deadline (slot j+1) still holds at 2 pops/slot
        for i, qb in enumerate(range(2, 7)):
            fillers.append(lambda qb=qb: gqk_item(qb))
            fillers.append(lambda bl=i: gv_block(bl))
        for bl in range(5, NKB):         # remaining b0 v blocks
            fillers.append(lambda bl=bl: gv_block(bl))
        for qb in range(7, 14):          # b1 qk projection (needed pair 3)
            fillers.append(lambda qb=qb: qk_proj(qb, gwqk, QKg))
        fillers.append(lambda: qk_dup(QKg, 1))
        for bl in range(NKB, 2 * NKB):   # b1 v blocks (needed pair 3)
            fillers.append(lambda bl=bl: gv_block(bl))
        for qb in range(14):             # local qk projection
            fillers.append(lambda qb=qb: qk_proj(qb, lwqk, QKl))
        fillers.append(lambda: qk_dup(QKl, 0))
        fillers.append(lambda: qk_dup(QKl, 1))
        for s in range(64):              # local v pairs
            fillers.append(lambda s=s: lv_pair(s))
        for r in range(8):               # local attention regions
            fillers.append(lambda r=r: local_region(r))
        # local projection: reserved tail work (DVE/DMA heavy, spread thin)
        tail_fillers = deque()
        for s0 in range(0, 64, 2):
            tail_fillers.append(lambda s0=s0: lproj_block(s0))

        total_slots = len(PAIRS) * (NKB + 1)
        slots_done = 0

        TAIL_START = 90

        def pop_fillers(force=None):
            nonlocal slots_done
            slots_done += 1
            rem_main = max(1, TAIL_START - slots_done)
            n = force if force is not None else min(2, max(1, -(-len(fillers) // rem_main)))
            for _ in range(n):
                if fillers:
                    fillers.popleft()()
            if not fillers or slots_done >= TAIL_START:
                rem_slots = max(1, total_slots - slots_done)
                nt = -(-len(tail_fillers) // rem_slots)
                for _ in range(nt):
                    if tail_fillers:
                        tail_fillers.popleft()()

        # ---- global attention loop ----
        for p, (bA, rA, bB, rB) in enumerate(PAIRS):
            qA0 = bA * N + rA * Q
            qB0 = bB * N + rB * Q
            po = psum.tile([128, Q], F32, name="po", tag="po", bufs=2)
            nc.vector.memset(po[:, :], 0.0)
            exs = [None] * NKB
            for j in range(NKB + 1):
                if j >= 1:
                    jj = j - 1
                    szj = 64 if jj == NKB - 1 else 128
                    ex = exs[jj]
                    blA = bA * NKB + jj
                    blB = bB * NKB + jj
                    stop = jj == NKB - 1
                    for (cA, vaA, exA) in ((0, blA, 0), (64, blB, Q)):
                        nc.tensor.matmul(po[cA:cA + 64, :],
                                         g_vaug[0:szj, vaA * VS:(vaA + 1) * VS],
                                         ex[0:szj, exA:exA + Q],
                                         start=False, stop=stop,
                                         skip_group_check=True)
                pop_fillers(force=2 if (p == 0 and j < 13) else None)
                if j < NKB:
                    sz = 64 if j == NKB - 1 else 128
                    kA = bA * N + j * 128
                    kB = bB * N + j * 128
                    ps = psum.tile([128, 1024], F32, name="pS", tag="pS", bufs=2)
                    # range A on partitions 0-47, range B on 64-111: the two S
                    # matmuls occupy different PE row groups -> concurrent
                    nc.tensor.matmul(ps[0:sz, 0:Q],
                                     QKg[0:48, NT + kA:NT + kA + sz],
                                     QKg[0:48, qA0:qA0 + Q],
                                     start=True, stop=True)
                    nc.tensor.matmul(ps[0:sz, 512:512 + Q],
                                     QKg[64:112, NT + kB:NT + kB + sz],
                                     QKg[64:112, qB0:qB0 + Q],
                                     start=True, stop=True)
                    ex = work.tile([128, 2 * Q], BF16, name="ex", tag="ex", bufs=6)
                    ps_v = ps[0:sz, :].rearrange("p (u k) -> p u k", k=512)[:, :, 0:Q]
                    ex_v = ex[0:sz, :].rearrange("p (u k) -> p u k", k=Q)
                    nc.scalar.activation(ex_v, ps_v, EXP, scale=SCALE)
                    exs[j] = ex

            # pair epilogue: unload outT + denominators
            pcol = p * Q
            nc.vector.tensor_copy(g_outT[0:48, pcol:pcol + Q], po[0:48, :])
            nc.vector.tensor_copy(g_outT[64:112, pcol:pcol + Q], po[64:112, :])
            dnA = work.tile([32, Q], F32, name="dnGA", tag="dn", bufs=4)
            dnB = work.tile([32, Q], F32, name="dnGB", tag="dn", bufs=4)
            nc.vector.tensor_copy(dnA[:, :], po[32:64, :])
            nc.vector.tensor_copy(dnB[:, :], po[96:128, :])
            nc.gpsimd.dma_start(dout["g_den"][0:1, qA0:qA0 + Q], dnA[31:32, :])
            nc.gpsimd.dma_start(dout["g_den"][0:1, qB0:qB0 + Q], dnB[31:32, :])
            # projection of this pair runs as filler during the next pair
            for t in range(4):
                fillers.appendleft(lambda p=p, t=t: gproj_block(p, t))

        # drain remaining fillers
        while fillers:
            fillers.popleft()()
        while tail_fillers:
            tail_fillers.popleft()()


def _host_prep(x, g_qkv_w, g_proj_w, l_qkv_w, l_proj_w):
    bf = ml_dtypes.bfloat16
    xf = np.asarray(x, np.float32).reshape(NT, C)
    xT = np.ascontiguousarray(xf.T).astype(bf)
    x4 = np.asarray(x, np.float32).reshape(B, 56, 56, C)
    win = x4.reshape(B, 8, WS, 8, WS, C).transpose(0, 1, 3, 5, 2, 4)
    win = win.reshape(B, 8, 8, WS, WS, C).transpose(0, 1, 2, 4, 3, 5).reshape(NT, C)
    winT = np.ascontiguousarray(win.T).astype(bf)

    in_maps = []
    for h in range(8):
        m = {"xT": xT, "winT": winT}
        for pre, qkv_w, proj_w in (("g", g_qkv_w, g_proj_w), ("l", l_qkv_w, l_proj_w)):
            qw = np.asarray(qkv_w[:, h * HD:(h + 1) * HD], np.float32)
            kw = np.asarray(qkv_w[:, C + h * HD:C + (h + 1) * HD], np.float32)
            vw = np.asarray(qkv_w[:, 2 * C + h * HD:2 * C + (h + 1) * HD], np.float32)
            wqk = np.zeros((C, 112), np.float32)
            wqk[:, 0:48] = qw
            wqk[:, 64:112] = kw
            m[pre + "wqk"] = wqk.astype(bf)
            m[pre + "wv"] = np.ascontiguousarray(vw).astype(bf)
            m[pre + "wp"] = np.ascontiguousarray(
                np.asarray(proj_w, np.float32)[h * HD:(h + 1) * HD, :]).astype(bf)
        in_maps.append(m)
    return in_maps


_NC_CACHE = None


def kernel(x, g_qkv_w, g_proj_w, g_proj_b, l_qkv_w, l_proj_w, l_proj_b):
    global _NC_CACHE
    if _NC_CACHE is None:
        _NC_CACHE = build_program()
    nc = _NC_CACHE

    in_maps = _host_prep(x, g_qkv_w, g_proj_w, l_qkv_w, l_proj_w)
    res = bass_utils.run_bass_kernel_spmd(nc, in_maps, core_ids=list(range(8)))

    acc = np.zeros((NT, C), np.float32)
    l_acc = np.zeros((NT, C), np.float32)
    for h in range(8):
        r = res.results[h]
        acc += np.asarray(r["g_out"], np.float32) / np.asarray(r["g_den"], np.float32).reshape(NT, 1)
        l_acc += np.asarray(r["l_out"], np.float32) / np.asarray(r["l_den"], np.float32).reshape(NT, 1)
    l_tok = l_acc.reshape(B, 8, 8, WS, WS, C).transpose(0, 1, 3, 2, 4, 5).reshape(NT, C)
    out = acc + l_tok + np.asarray(g_proj_b, np.float32) + np.asarray(l_proj_b, np.float32)
    return out.reshape(B, N, C).astype(np.float32)


# revision 31
# speedup vs baseline: 1.0045x; 1.0045x over previous
"""Trainium2 Bass kernel for LGAttention (global MHA + windowed local MHA).

Sharding: one attention head per NeuronCore (8 heads, 8 cores), SPMD.

v2 design (vs baseline): the kernel is paced by the ScalarE exp of the full
attention matrix; everything else hides under it.
  - Global S matmuls (contraction hd=48) are row-packed: qT/kT live at
    partitions 0-47 AND (duplicated via SBUF->SBUF DMA) 64-111; k-blocks of
    alternating parity use alternating row groups so consecutive S matmuls run
    concurrently in the PE array.
  - PV matmuls are col-packed: v_aug is 64 wide (48 v + pad + ones col at 63,
    so the softmax denominator lands on out-partition 63/127); two q-ranges
    are processed per pair with accumulators at out partitions 0-63 / 64-127
    of one PSUM bank (memset + start=False accumulation).
  - exp is one ACT instruction per k-block covering both q-ranges (896 wide).
  - Local branch: window pairs stacked at partitions 0-48 / 64-112
    (diagonal-tiled S and PV), exp batched 16 windows per instruction.
  - Projections row-packed (outT for even/odd ranges at partitions 0-47 /
    64-111, proj weights duplicated) and interleaved as PE filler.
Host: divides by the denominators, un-permutes the local branch, sums the
8 per-head partials, adds biases (same contract as baseline).
"""

import sys

sys.path.insert(0, "/opt/trn_rl_repo")

from collections import deque

import numpy as np
import ml_dtypes

import concourse.bass as bass
import concourse.mybir as mybir
import concourse.tile as tile
from concourse import bacc, bass_utils

BF16 = mybir.dt.bfloat16
F32 = mybir.dt.float32

B, N, C = 2, 3136, 384
H, HD, WS = 8, 48, 7
NT = B * N            # 6272 tokens total
WT = WS * WS          # 49 tokens per window
Q = 448               # q-range width; 7 ranges per batch
VS = 64               # v_aug column stride: 48 v + 15 pad + 1 ones (col 63)
SCALE = float(HD) ** -0.5
NKB = 25              # k-blocks per batch (24x128 + 64)
EXP = mybir.ActivationFunctionType.Exp

# global q-range pairs: (batch_A, range_A, batch_B, range_B)
PAIRS = [(0, 0, 0, 1), (0, 2, 0, 3), (0, 4, 0, 5),
         (1, 0, 1, 1), (1, 2, 1, 3), (1, 4, 1, 5),
         (0, 6, 1, 6)]


def build_program():
    nc = bacc.Bacc(
        "TRN2",
        target_bir_lowering=False,
        debug=False,
        enable_asserts=False,
        num_devices=8,
    )

    din = {}
    for name, shape in [
        ("xT", (C, NT)), ("winT", (C, NT)),
        ("gwqk", (C, 112)), ("gwv", (C, HD)), ("gwp", (HD, C)),
        ("lwqk", (C, 112)), ("lwv", (C, HD)), ("lwp", (HD, C)),
    ]:
        din[name] = nc.dram_tensor(name, list(shape), BF16, kind="ExternalInput").ap()

    dout = {}
    for name, shape in [
        ("g_out", (NT, C)), ("l_out", (NT, C)),
        ("g_den", (1, NT)), ("l_den", (1, NT)),
    ]:
        dout[name] = nc.dram_tensor(name, list(shape), F32, kind="ExternalOutput").ap()

    with tile.TileContext(nc) as tc:
        _emit(tc, nc, din, dout)

    nc.compile()
    return nc


def _emit(tc, nc, din, dout):
    from contextlib import ExitStack

    ctx = ExitStack()
    with ctx:
        persist = ctx.enter_context(tc.tile_pool(name="persist", bufs=1))
        psum = ctx.enter_context(tc.tile_pool(name="psum", bufs=1, space="PSUM"))
        work = ctx.enter_context(tc.tile_pool(name="work", bufs=1))

        # ---- persistent tiles ----
        xt = [persist.tile([128, NT], BF16, name=f"xt{c}") for c in range(3)]
        wt = [persist.tile([128, NT], BF16, name=f"wt{c}") for c in range(3)]
        gwqk = persist.tile([128, 3 * 112], BF16, name="gwqk")
        lwqk = persist.tile([128, 3 * 112], BF16, name="lwqk")
        gwv = persist.tile([128, 3 * 48], BF16, name="gwv")
        lwv = persist.tile([128, 3 * 48], BF16, name="lwv")
        gwp_d = persist.tile([128, C], BF16, name="gwp_d")
        lwp_d = persist.tile([128, C], BF16, name="lwp_d")
        # q/k in transposed layout, duplicated on partitions 0-47 and 64-111:
        # cols [0,NT) = qT (token t at col t), cols [NT,2NT) = kT, + 64 pad
        # cols (local S reads 64-wide lhsT slices that overrun the last window)
        QKg = persist.tile([128, 2 * NT + 64], BF16, name="QKg")
        QKl = persist.tile([128, 2 * NT + 64], BF16, name="QKl")
        g_vaug = persist.tile([128, 2 * NKB * VS], BF16, name="g_vaug")
        l_vaug = persist.tile([128, 64 * VS], BF16, name="l_vaug")
        # outT: rows 0-47 = even q-range (A), rows 64-111 = odd q-range (B);
        # pair p occupies cols [448p, 448p+448)
        g_outT = persist.tile([128, 7 * Q], BF16, name="g_outT")
        # local: rows 0-47 = even window, 64-111 = odd window; win pair s at
        # cols [49s, 49s+49)
        l_outT = persist.tile([128, 64 * WT], BF16, name="l_outT")

        # ---- weight + input DMAs ----
        for c in range(3):
            nc.scalar.dma_start(gwqk[:, c * 112:(c + 1) * 112], din["gwqk"][c * 128:(c + 1) * 128, :])
            nc.scalar.dma_start(lwqk[:, c * 112:(c + 1) * 112], din["lwqk"][c * 128:(c + 1) * 128, :])
            nc.scalar.dma_start(gwv[:, c * 48:(c + 1) * 48], din["gwv"][c * 128:(c + 1) * 128, :])
            nc.scalar.dma_start(lwv[:, c * 48:(c + 1) * 48], din["lwv"][c * 128:(c + 1) * 128, :])
        nc.scalar.dma_start(gwp_d[0:48, :], din["gwp"][:, :])
        nc.scalar.dma_start(gwp_d[64:112, :], din["gwp"][:, :])
        nc.scalar.dma_start(lwp_d[0:48, :], din["lwp"][:, :])
        nc.scalar.dma_start(lwp_d[64:112, :], din["lwp"][:, :])

        # dummy exp to pull the ACT table load into the DMA phase
        dmy = work.tile([1, 16], F32, name="dmy", tag="dmy", bufs=1)
        dmyo = work.tile([1, 16], BF16, name="dmyo", tag="dmy2", bufs=1)
        nc.vector.memset(dmy[:, :], 0.0)
        nc.scalar.activation(dmyo[:, :], dmy[:, :], EXP, scale=SCALE)

        # input DMAs: one half-tensor (contiguous DRAM rows) per c-chunk,
        # spread across three DMA queues (sync / gpsimd / vector) so the
        # first qk-proj can start after ~one transfer
        qengs = [nc.sync, nc.scalar, nc.sync]
        for lo in range(0, N, 896):      # fine-grained chunks for fast ramp
            for c in range(3):
                hi = min(lo + 896, N)
                qengs[c].dma_start(xt[c][:, lo:hi], din["xT"][c * 128:(c + 1) * 128, lo:hi])
        for c in range(3):
            qengs[c].dma_start(xt[c][:, N:NT], din["xT"][c * 128:(c + 1) * 128, N:NT])
        for lo in (0, N):
            for c in range(3):
                qengs[c].dma_start(wt[c][:, lo:lo + N], din["winT"][c * 128:(c + 1) * 128, lo:lo + N])

        # ---- pad/ones init ----
        for vaug, nsl in ((g_vaug, 2 * NKB), (l_vaug, 64)):
            v3 = vaug[:, :].rearrange("p (s k) -> p s k", k=VS)
            nc.vector.memset(v3[:, :, 48:VS], 0.0)
            nc.vector.memset(v3[:, :, 63:VS], 1.0)
        nc.vector.memset(QKg[:, 2 * NT:], 0.0)
        nc.vector.memset(QKl[:, 2 * NT:], 0.0)

        # ---- qk projection emitter (writes qT to rows 0-47, kT to 64-111) ----
        def qk_proj(qb, wqk, QK):
            t0 = qb * Q
            ps = psum.tile([112, Q], F32, name="pqk", tag="pmix", bufs=2)
            src = xt if QK is QKg else wt
            for c in range(3):
                nc.tensor.matmul(ps[:, :], wqk[:, c * 112:(c + 1) * 112],
                                 src[c][:, t0:t0 + Q], start=(c == 0), stop=(c == 2))
            nc.vector.tensor_copy(QK[0:48, t0:t0 + Q], ps[0:48, :])
            nc.vector.tensor_copy(QK[64:112, NT + t0:NT + t0 + Q], ps[64:112, :])

        # duplicate q (lo->hi) and k (hi->lo) for one batch via SBUF->SBUF DMA
        def qk_dup(QK, b):
            t0 = b * N
            nc.sync.dma_start(QK[64:112, t0:t0 + N], QK[0:48, t0:t0 + N])
            nc.sync.dma_start(QK[0:48, NT + t0:NT + t0 + N], QK[64:112, NT + t0:NT + t0 + N])

        # ---- global v projection: one 128-token block, token-major ----
        def gv_block(bl):
            t0 = (bl // NKB) * N + (bl % NKB) * 128
            sz = 64 if bl % NKB == NKB - 1 else 128
            pv = psum.tile([128, 48], F32, name="pv", tag="pmix", bufs=2)
            for c in range(3):
                nc.tensor.matmul(pv[0:sz, :], xt[c][:, t0:t0 + sz],
                                 gwv[:, c * 48:(c + 1) * 48], start=(c == 0), stop=(c == 2))
            nc.vector.tensor_copy(g_vaug[0:sz, bl * VS:bl * VS + 48], pv[0:sz, :])

        # ---- local v projection: one window pair (A rows 0-48, B rows 64-112) ----
        def lv_pair(s):
            tA = (2 * s) * WT
            tB = (2 * s + 1) * WT
            pvl = psum.tile([128, 48], F32, name="pvl", tag="pmix", bufs=2)
            nc.vector.memset(pvl[:, :], 0.0)
            for c in range(3):
                nc.tensor.matmul(pvl[0:WT, :], wt[c][:, tA:tA + WT],
                                 lwv[:, c * 48:(c + 1) * 48], start=False, stop=(c == 2),
                                 skip_group_check=True)
            for c in range(3):
                nc.tensor.matmul(pvl[64:64 + WT, :], wt[c][:, tB:tB + WT],
                                 lwv[:, c * 48:(c + 1) * 48], start=False, stop=(c == 2),
                                 skip_group_check=True)
            nc.vector.tensor_copy(l_vaug[:, s * VS:s * VS + 48], pvl[:, :])

        # ---- one local region: 8 window pairs (16 windows) ----
        def local_region(r):
            psl = psum.tile([128, 8 * WT], F32, name="psl", tag="pmix", bufs=2)
            for u in range(8):
                s = 8 * r + u
                tA, tB = (2 * s) * WT, (2 * s + 1) * WT
                # lhsT free padded to 64 (overruns into next window / pad cols)
                nc.tensor.matmul(psl[0:64, u * WT:(u + 1) * WT],
                                 QKl[0:48, NT + tA:NT + tA + 64],
                                 QKl[0:48, tA:tA + WT], start=True, stop=True)
                nc.tensor.matmul(psl[64:128, u * WT:(u + 1) * WT],
                                 QKl[64:112, NT + tB:NT + tB + 64],
                                 QKl[64:112, tB:tB + WT], start=True, stop=True)
            exl = work.tile([128, 8 * WT], BF16, name="exl", tag="exl", bufs=3)
            nc.scalar.activation(exl[:, :], psl[:, :], EXP, scale=SCALE)
            pol = psum.tile([128, 8 * WT], F32, name="pol", tag="pmix", bufs=2)
            for u in range(8):
                s = 8 * r + u
                nc.tensor.matmul(pol[0:64, u * WT:(u + 1) * WT],
                                 l_vaug[0:WT, s * VS:(s + 1) * VS],
                                 exl[0:WT, u * WT:(u + 1) * WT], start=True, stop=True)
                nc.tensor.matmul(pol[64:128, u * WT:(u + 1) * WT],
                                 l_vaug[64:64 + WT, s * VS:(s + 1) * VS],
                                 exl[64:64 + WT, u * WT:(u + 1) * WT], start=True, stop=True)
            nc.vector.tensor_copy(l_outT[0:48, r * 392:(r + 1) * 392], pol[0:48, :])
            nc.vector.tensor_copy(l_outT[64:112, r * 392:(r + 1) * 392], pol[64:112, :])
            dnA = work.tile([32, 392], F32, name="dnA", tag="dn", bufs=4)
            dnB = work.tile([32, 392], F32, name="dnB", tag="dn", bufs=4)
            nc.vector.tensor_copy(dnA[:, :], pol[32:64, :])
            nc.vector.tensor_copy(dnB[:, :], pol[96:128, :])
            ld = dout["l_den"].rearrange("o (a b t) -> o a b t", b=2, t=WT)
            nc.gpsimd.dma_start(ld[0:1, 8 * r:8 * r + 8, 0, :],
                              dnA[31:32, :].rearrange("o (a t) -> o a t", t=WT))
            nc.gpsimd.dma_start(ld[0:1, 8 * r:8 * r + 8, 1, :],
                              dnB[31:32, :].rearrange("o (a t) -> o a t", t=WT))

        # ---- projection block emitters ----
        def gproj_block(p, t):
            # one 112-token block of pair p, both ranges (adjacent row-tiled
            # matmuls on row groups 0 / 64 -> concurrent)
            bA, rA, bB, rB = PAIRS[p]
            col = p * Q + t * 112
            pps = []
            for hi, q0 in ((0, bA * N + rA * Q), (1, bB * N + rB * Q)):
                rg = 64 * hi
                pp = psum.tile([112, C], F32, name="pp", tag="pmix", bufs=2)
                nc.tensor.matmul(pp[:, :], g_outT[rg:rg + 48, col:col + 112],
                                 gwp_d[rg:rg + 48, :], start=True, stop=True)
                pps.append((pp, q0))
            for pp, q0 in pps:
                sp = work.tile([112, C], F32, name="sp", tag="sp", bufs=5)
                nc.vector.tensor_copy(sp[:, :], pp[:, :])
                nc.sync.dma_start(dout["g_out"][q0 + t * 112:q0 + t * 112 + 112, :], sp[:, :])

        def lproj_block(s0):
            # windows 2*s0 .. 2*s0+3 (both parities, row-tiled concurrent)
            col = s0 * WT
            pps = []
            for hi in (0, 1):
                rg = 64 * hi
                pp = psum.tile([98, C], F32, name="ppl", tag="pmix", bufs=2)
                nc.tensor.matmul(pp[:, :], l_outT[rg:rg + 48, col:col + 98],
                                 lwp_d[rg:rg + 48, :], start=True, stop=True)
                pps.append((pp, hi))
            for pp, hi in pps:
                sp = work.tile([98, C], F32, name="spl", tag="sp", bufs=5)
                nc.vector.tensor_copy(sp[:, :], pp[:, :])
                r0 = (2 * s0 + hi) * WT
                r1 = (2 * (s0 + 1) + hi) * WT
                nc.sync.dma_start(dout["l_out"][r0:r0 + WT, :], sp[0:WT, :])
                nc.gpsimd.dma_start(dout["l_out"][r1:r1 + WT, :], sp[WT:2 * WT, :])

        # ---- HAM pre-warm: dense dummy matmuls while input DMAs stream ----
        # (the PE is otherwise idle here; ~6us of back-to-back matmuls pushes
        # the activity monitor to K=8/8 before the real work begins)
        pwm = psum.tile([112, 448], F32, name="pwm", tag="pmix", bufs=2)
        for _ in range(16):
            nc.tensor.matmul(pwm[:, :336], gwqk[:, 0:112], gwqk[:, 0:336],
                             start=True, stop=True)

        # ---- b0 qk projection + per-chunk dup ----
        def gqk_item(qb):
            qk_proj(qb, gwqk, QKg)
            t0 = qb * Q
            nc.sync.dma_start(QKg[64:112, t0:t0 + Q], QKg[0:48, t0:t0 + Q])
            nc.sync.dma_start(QKg[0:48, NT + t0:NT + t0 + Q], QKg[64:112, NT + t0:NT + t0 + Q])

        for qb in range(7):
            gqk_item(qb)

        # ---- filler queue for the global loop ----
        fillers = deque()
        for bl in range(NKB):            # b0 v blocks (needed from pair 0)
            fillers.append(lambda bl=bl: gv_block(bl))
        for qb in range(7, 14):          # b1 qk projection (needed pair 3)
            fillers.append(lambda qb=qb: qk_proj(qb, gwqk, QKg))
        fillers.append(lambda: qk_dup(QKg, 1))
        for bl in range(NKB, 2 * NKB):   # b1 v blocks (needed pair 3)
            fillers.append(lambda bl=bl: gv_block(bl))
        for qb in range(14):             # local qk projection
            fillers.append(lambda qb=qb: qk_proj(qb, lwqk, QKl))
        fillers.append(lambda: qk_dup(QKl, 0))
        fillers.append(lambda: qk_dup(QKl, 1))
        for s in range(64):              # local v pairs
            fillers.append(lambda s=s: lv_pair(s))
        for r in range(8):               # local attention regions
            fillers.append(lambda r=r: local_region(r))
        # local projection: reserved tail work (DVE/DMA heavy, spread thin)
        tail_fillers = deque()
        for s0 in range(0, 64, 2):
            tail_fillers.append(lambda s0=s0: lproj_block(s0))

        total_slots = len(PAIRS) * (NKB + 1)
        slots_done = 0

        TAIL_START = 90

        def pop_fillers(force=None):
            nonlocal slots_done
            slots_done += 1
            rem_main = max(1, TAIL_START - slots_done)
            n = force if force is not None else min(2, max(1, -(-len(fillers) // rem_main)))
            for _ in range(n):
                if fillers:
                    fillers.popleft()()
            if not fillers or slots_done >= TAIL_START:
                rem_slots = max(1, total_slots - slots_done)
                nt = -(-len(tail_fillers) // rem_slots)
                for _ in range(nt):
                    if tail_fillers:
                        tail_fillers.popleft()()

        # ---- global attention loop ----
        for p, (bA, rA, bB, rB) in enumerate(PAIRS):
            qA0 = bA * N + rA * Q
            qB0 = bB * N + rB * Q
            po = psum.tile([128, Q], F32, name="po", tag="po", bufs=2)
            nc.vector.memset(po[:, :], 0.0)
            exs = [None] * NKB
            for j in range(NKB + 1):
                if j >= 1:
                    jj = j - 1
                    szj = 64 if jj == NKB - 1 else 128
                    ex = exs[jj]
                    blA = bA * NKB + jj
                    blB = bB * NKB + jj
                    stop = jj == NKB - 1
                    for (cA, vaA, exA) in ((0, blA, 0), (64, blB, Q)):
                        nc.tensor.matmul(po[cA:cA + 64, :],
                                         g_vaug[0:szj, vaA * VS:(vaA + 1) * VS],
                                         ex[0:szj, exA:exA + Q],
                                         start=False, stop=stop,
                                         skip_group_check=True)
                pop_fillers(force=2 if (p == 0 and j < 13) else None)
                if j < NKB:
                    sz = 64 if j == NKB - 1 else 128
                    kA = bA * N + j * 128
                    kB = bB * N + j * 128
                    ps = psum.tile([128, 1024], F32, name="pS", tag="pS", bufs=2)
                    # range A on partitions 0-47, range B on 64-111: the two S
                    # matmuls occupy different PE row groups -> concurrent
                    nc.tensor.matmul(ps[0:sz, 0:Q],
                                     QKg[0:48, NT + kA:NT + kA + sz],
                                     QKg[0:48, qA0:qA0 + Q],
                                     start=True, stop=True)
                    nc.tensor.matmul(ps[0:sz, 512:512 + Q],
                                     QKg[64:112, NT + kB:NT + kB + sz],
                                     QKg[64:112, qB0:qB0 + Q],
                                     start=True, stop=True)
                    ex = work.tile([128, 2 * Q], BF16, name="ex", tag="ex", bufs=6)
                    ps_v = ps[0:sz, :].rearrange("p (u k) -> p u k", k=512)[:, :, 0:Q]
                    ex_v = ex[0:sz, :].rearrange("p (u k) -> p u k", k=Q)
                    nc.scalar.activation(ex_v, ps_v, EXP, scale=SCALE)
                    exs[j] = ex

            # pair epilogue: unload outT + denominators
            pcol = p * Q
            nc.vector.tensor_copy(g_outT[0:48, pcol:pcol + Q], po[0:48, :])
            nc.vector.tensor_copy(g_outT[64:112, pcol:pcol + Q], po[64:112, :])
            dnA = work.tile([32, Q], F32, name="dnGA", tag="dn", bufs=4)
            dnB = work.tile([32, Q], F32, name="dnGB", tag="dn", bufs=4)
            nc.vector.tensor_copy(dnA[:, :], po[32:64, :])
            nc.vector.tensor_copy(dnB[:, :], po[96:128, :])
            nc.gpsimd.dma_start(dout["g_den"][0:1, qA0:qA0 + Q], dnA[31:32, :])
            nc.gpsimd.dma_start(dout["g_den"][0:1, qB0:qB0 + Q], dnB[31:32, :])
            # projection of this pair runs as filler during the next pair
            for t in range(4):
                fillers.appendleft(lambda p=p, t=t: gproj_block(p, t))

        # drain remaining fillers
        while fillers:
            fillers.popleft()()
        while tail_fillers:
            tail_fillers.popleft()()


def _host_prep(x, g_qkv_w, g_proj_w, l_qkv_w, l_proj_w):
    bf = ml_dtypes.bfloat16
    xf = np.asarray(x, np.float32).reshape(NT, C)
    xT = np.ascontiguousarray(xf.T).astype(bf)
    x4 = np.asarray(x, np.float32).reshape(B, 56, 56, C)
    win = x4.reshape(B, 8, WS, 8, WS, C).transpose(0, 1, 3, 5, 2, 4)
    win = win.reshape(B, 8, 8, WS, WS, C).transpose(0, 1, 2, 4, 3, 5).reshape(NT, C)
    winT = np.ascontiguousarray(win.T).astype(bf)

    in_maps = []
    for h in range(8):
        m = {"xT": xT, "winT": winT}
        for pre, qkv_w, proj_w in (("g", g_qkv_w, g_proj_w), ("l", l_qkv_w, l_proj_w)):
            qw = np.asarray(qkv_w[:, h * HD:(h + 1) * HD], np.float32)
            kw = np.asarray(qkv_w[:, C + h * HD:C + (h + 1) * HD], np.float32)
            vw = np.asarray(qkv_w[:, 2 * C + h * HD:2 * C + (h + 1) * HD], np.float32)
            wqk = np.zeros((C, 112), np.float32)
            wqk[:, 0:48] = qw
            wqk[:, 64:112] = kw
            m[pre + "wqk"] = wqk.astype(bf)
            m[pre + "wv"] = np.ascontiguousarray(vw).astype(bf)
            m[pre + "wp"] = np.ascontiguousarray(
                np.asarray(proj_w, np.float32)[h * HD:(h + 1) * HD, :]).astype(bf)
        in_maps.append(m)
    return in_maps


_NC_CACHE = None


def kernel(x, g_qkv_w, g_proj_w, g_proj_b, l_qkv_w, l_proj_w, l_proj_b):
    global _NC_CACHE
    if _NC_CACHE is None:
        _NC_CACHE = build_program()
    nc = _NC_CACHE

    in_maps = _host_prep(x, g_qkv_w, g_proj_w, l_qkv_w, l_proj_w)
    res = bass_utils.run_bass_kernel_spmd(nc, in_maps, core_ids=list(range(8)))

    acc = np.zeros((NT, C), np.float32)
    l_acc = np.zeros((NT, C), np.float32)
    for h in range(8):
        r = res.results[h]
        acc += np.asarray(r["g_out"], np.float32) / np.asarray(r["g_den"], np.float32).reshape(NT, 1)
        l_acc += np.asarray(r["l_out"], np.float32) / np.asarray(r["l_den"], np.float32).reshape(NT, 1)
    l_tok = l_acc.reshape(B, 8, 8, WS, WS, C).transpose(0, 1, 3, 2, 4, 5).reshape(NT, C)
    out = acc + l_tok + np.asarray(g_proj_b, np.float32) + np.asarray(l_proj_b, np.float32)
    return out.reshape(B, N, C).astype(np.float32)
